# revision 60
# baseline (speedup 1.0000x reference)
"""Trainium2 Bass kernel for nn_BSplineActivation.

Math: y[b,f] = sum_n B_n(x[b,f]) * coeff[f,n] with cubic B-spline bases on a
uniform grid linspace(-1,1,14). Using the truncated-power identity
  M3(v) = (1/6) sum_r (-1)^r C(4,r) (v-r)_+^3
the whole activation collapses to
  y = sum_{j=0}^{12} d_j[f] * relu(u - j)^3,   u = 6.5*clip(x,-1,1) + 6.5
with d_j[f] = (1/6) sum_r (-1)^r C(4,r) coeff[f, j-r].

Per j:  R_j = relu(u - j),  S_j = R_j^2 (ACT Square), then with
  y = (u - 6.5)*A + C,  A = sum d_j S_j,  C = sum (6.5-j) d_j S_j
the (A, C) accumulation chains are split by data chunk: PE chunks use
diagonal-matmul PSUM accumulation (lhsT = diag(d_j), exact fp32), the rest
use DVE scalar_tensor_tensor chains with per-partition scalar columns.
R_j producers are balanced across POOL/ACT; diag matrices are built by POOL
affine_select from a packed per-partition table; the span tail add
(Y += Cacc) rides a SWDGE DMA with destination-accumulate, freeing DVE.

Device layout: features on partitions (128 per group, 8 groups/core), batch
along the free dim; pure data parallel over batch across 8 cores. The host
passes per-core batch shards transposed (features-major) so all DMAs are
burst-friendly; total HBM bytes moved are identical to the untransposed
layout.
"""

import os
from math import comb

import numpy as np

import concourse.bacc as bacc
import concourse.bass as bass
import concourse.mybir as mybir
import concourse.tile as tile
from concourse.bass_utils import run_bass_kernel_spmd

N_CORES = 8
B_FULL, F = 8192, 1024
B_CORE = B_FULL // N_CORES  # 1024
NB = 13
P = 128
G = F // P  # 8
QUARTERS = 4
CHUNK = 512
FP32 = mybir.dt.float32

Alu = mybir.AluOpType
Act = mybir.ActivationFunctionType

# PE owns chunks [0, PE_SPAN); the rest is the DVE dual-chain span.
PE_CHUNKS = [(0, 0), (0, 1)]  # contiguous span 0..1024
PE_SPAN = 2 * CHUNK
# chunk (0,1): j < MIX_SPLIT accumulate on PE, j >= MIX_SPLIT on DVE
# (partial sums merged in the tail).
MIX_SPLIT = 11
# R_j producer per j: "dve" | "pool" | "act" (R_0 = u is skipped entirely)
RENG = {
    0: "pool", 1: "pool", 2: "pool", 3: "pool", 4: "pool", 5: "pool",
    6: "pool", 7: "pool", 8: "pool", 9: "act", 10: "act", 11: "act", 12: "act",
}
_CACHE: dict = {}


def _build_nc() -> bass.Bass:
    nc = bacc.Bacc("TRN2", target_bir_lowering=False, debug=False)

    xT = nc.dram_tensor("xT", [F, B_CORE], FP32, kind="ExternalInput")
    # packed tables: cols [0, G*NB) = d_j; [G*NB, 2*G*NB) = c_j = (6.5-j)*d_j;
    # cols [2*G*NB, 2*G*NB+NB) = constant -j (ACT relu bias columns)
    tabs = nc.dram_tensor("tabs", [P, 2 * G * NB + NB], FP32, kind="ExternalInput")
    yT = nc.dram_tensor("yT", [F, B_CORE], FP32, kind="ExternalOutput")

    W = 2 * B_CORE

    with tile.TileContext(nc) as tc:
        with (
            tc.tile_pool(name="const", bufs=1) as const_pool,
            tc.tile_pool(name="xdata", bufs=2) as x_pool,
            tc.tile_pool(name="rs", bufs=4) as rs_pool,
            tc.tile_pool(name="yout", bufs=2) as y_pool,
            tc.tile_pool(name="diag", bufs=2) as diag_pool,
            tc.tile_pool(name="psum", bufs=2, space="PSUM") as psum_pool,
        ):
            tabs_t = const_pool.tile([P, 2 * G * NB + NB], FP32, name="tabs_t")
            nc.sync.dma_start(tabs_t[:], tabs[:])

            def dcol(g, j):
                return tabs_t[:, g * NB + j : g * NB + j + 1]

            def ccol(g, j):
                c = G * NB + g * NB + j
                return tabs_t[:, c : c + 1]

            def bcol(j):
                c = 2 * G * NB + j
                return tabs_t[:, c : c + 1]

            for q in range(QUARTERS):
                g0 = 2 * q
                X = x_pool.tile([P, W], FP32, name="X", tag="X")
                nc.sync.dma_start(
                    X[:].rearrange("p (gl b) -> p gl b", gl=2),
                    xT[g0 * P : (g0 + 2) * P, :].rearrange("(gl p) b -> p gl b", p=P),
                )
                nc.vector.tensor_scalar(X[:], X[:], -1.0, 1.0, Alu.max, Alu.min)
                nc.vector.tensor_scalar(X[:], X[:], 6.5, 6.5, Alu.mult, Alu.add)

                pe_gls = sorted({gl for (gl, ch) in PE_CHUNKS})
                diagsA = {}
                diagsC = {}
                for gl in pe_gls:
                    g = g0 + gl
                    for j in range(NB):
                        dA = diag_pool.tile(
                            [P, P], FP32, name=f"dA{gl}_{j}", tag=f"dA{gl}_{j}"
                        )
                        dC = diag_pool.tile(
                            [P, P], FP32, name=f"dC{gl}_{j}", tag=f"dC{gl}_{j}"
                        )
                        nc.gpsimd.affine_select(
                            dA[:], dcol(g, j).broadcast_to([P, P]),
                            pattern=[[-1, P]], compare_op=Alu.is_equal,
                            fill=0.0, base=0, channel_multiplier=1,
                        )
                        nc.gpsimd.affine_select(
                            dC[:], ccol(g, j).broadcast_to([P, P]),
                            pattern=[[-1, P]], compare_op=Alu.is_equal,
                            fill=0.0, base=0, channel_multiplier=1,
                        )
                        diagsA[(gl, j)] = dA
                        diagsC[(gl, j)] = dC

                Y = y_pool.tile([P, W], FP32, name="Y", tag="Y")
                Apsum = {}
                Cpsum = {}
                for key in PE_CHUNKS:
                    gl, ch = key
                    Apsum[key] = psum_pool.tile(
                        [P, CHUNK], FP32, name=f"Yp{gl}{ch}", tag=f"Yp{gl}{ch}"
                    )
                    Cpsum[key] = psum_pool.tile(
                        [P, CHUNK], FP32, name=f"Cq{gl}{ch}", tag=f"Cq{gl}{ch}"
                    )
                # dual-chain accumulators for the DVE span [PE_SPAN, W)
                DW = W - PE_SPAN
                Aacc = y_pool.tile([P, DW], FP32, name="Aacc", tag="Aacc")
                Cacc = y_pool.tile([P, DW], FP32, name="Cacc", tag="Cacc")
                # DVE partials for the mixed chunk (0,1), j >= MIX_SPLIT
                Amix = y_pool.tile([P, CHUNK], FP32, name="Amix", tag="Amix")
                Cmix = y_pool.tile([P, CHUNK], FP32, name="Cmix", tag="Cmix")

                for j in range(NB):
                    if j == 0:
                        R = X  # relu(u - 0) = u since u >= 0
                    else:
                        R = rs_pool.tile([P, W], FP32, name="R", tag="R")
                        reng = RENG[j]
                        if reng == "dve":
                            nc.vector.tensor_scalar(
                                R[:], X[:], float(-j), 0.0, Alu.add, Alu.max
                            )
                        elif reng == "pool":
                            nc.gpsimd.tensor_scalar(
                                R[:], X[:], float(-j), 0.0, Alu.add, Alu.max
                            )
                        else:
                            nc.scalar.activation(
                                R[:], X[:], Act.Relu, bias=bcol(j), scale=1.0
                            )
                    S = rs_pool.tile([P, W], FP32, name="S", tag="S")
                    nc.scalar.activation(S[:], R[:], Act.Square)
                    for gl, ch in PE_CHUNKS:
                        if (gl, ch) == (0, 1) and j >= MIX_SPLIT:
                            continue  # handled by the DVE mix chain below
                        lo = gl * B_CORE + ch * CHUNK
                        last = (j == NB - 1) if (gl, ch) != (0, 1) else (
                            j == MIX_SPLIT - 1
                        )
                        nc.tensor.matmul(
                            Apsum[(gl, ch)][:], diagsA[(gl, j)][:],
                            S[:, lo : lo + CHUNK],
                            start=(j == 0), stop=last,
                        )
                        nc.tensor.matmul(
                            Cpsum[(gl, ch)][:], diagsC[(gl, j)][:],
                            S[:, lo : lo + CHUNK],
                            start=(j == 0), stop=last,
                        )
                    if j >= MIX_SPLIT:
                        # DVE partial for mixed chunk (0,1): features of gl=0
                        g = g0
                        msl = S[:, CHUNK : 2 * CHUNK]
                        if j == MIX_SPLIT:
                            nc.vector.tensor_scalar(
                                Amix[:], msl, dcol(g, j), None, Alu.mult
                            )
                            nc.vector.tensor_scalar(
                                Cmix[:], msl, ccol(g, j), None, Alu.mult
                            )
                        else:
                            nc.vector.scalar_tensor_tensor(
                                Amix[:], msl, dcol(g, j), Amix[:], Alu.mult, Alu.add
                            )
                            nc.vector.scalar_tensor_tensor(
                                Cmix[:], msl, ccol(g, j), Cmix[:], Alu.mult, Alu.add
                            )
                    # dual chain on S for the tail span (features of gl=1)
                    g = g0 + 1
                    ssl = S[:, PE_SPAN:W]
                    if j == 0:
                        nc.vector.tensor_scalar(
                            Aacc[:], ssl, dcol(g, j), None, Alu.mult
                        )
                        nc.vector.tensor_scalar(
                            Cacc[:], ssl, ccol(g, j), None, Alu.mult
                        )
                    else:
                        nc.vector.scalar_tensor_tensor(
                            Aacc[:], ssl, dcol(g, j), Aacc[:], Alu.mult, Alu.add
                        )
                        nc.vector.scalar_tensor_tensor(
                            Cacc[:], ssl, ccol(g, j), Cacc[:], Alu.mult, Alu.add
                        )
                # merge mixed-chunk DVE partials into its psum result
                # (DVE: GPSIMD has no PSUM port)
                nc.vector.tensor_tensor(
                    Amix[:], Amix[:], Apsum[(0, 1)][:], Alu.add
                )
                nc.vector.tensor_tensor(
                    Cmix[:], Cmix[:], Cpsum[(0, 1)][:], Alu.add
                )

                # tail: PE chunks drain psum -> Y on ACT; DVE span computes
                # y = (u - 6.5) * A + C in place into Y
                # fused tail: Y = (X - 6.5) * A in one scalar_tensor_tensor
                for gl, ch in PE_CHUNKS:
                    lo = gl * B_CORE + ch * CHUNK
                    Afin = Amix[:] if (gl, ch) == (0, 1) else Apsum[(gl, ch)][:]
                    Cfin = Cmix[:] if (gl, ch) == (0, 1) else Cpsum[(gl, ch)][:]
                    nc.vector.scalar_tensor_tensor(
                        Y[:, lo : lo + CHUNK], X[:, lo : lo + CHUNK],
                        -6.5, Afin, Alu.add, Alu.mult,
                    )
                    nc.vector.tensor_tensor(
                        Y[:, lo : lo + CHUNK], Y[:, lo : lo + CHUNK],
                        Cfin, Alu.add,
                    )
                nc.vector.scalar_tensor_tensor(
                    Y[:, PE_SPAN:W], X[:, PE_SPAN:W],
                    -6.5, Aacc[:], Alu.add, Alu.mult,
                )
                # Y += Cacc via SWDGE destination-accumulate (offloads DVE)
                nc.gpsimd.dma_start(
                    Y[:, PE_SPAN:W], Cacc[:], accum_op=Alu.add
                )

                nc.sync.dma_start(
                    yT[g0 * P : (g0 + 2) * P, :].rearrange("(gl p) b -> p gl b", p=P),
                    Y[:].rearrange("p (gl b) -> p gl b", gl=2),
                )
    nc.compile()
    return nc


def _tables(coeff: np.ndarray):
    """Packed [P, 2*G*NB + NB] fp32 table: d_j, c_j, -j bias columns."""
    d = np.zeros((NB, F), dtype=np.float64)
    c64 = coeff.astype(np.float64)
    for j in range(NB):
        for r in range(5):
            n = j - r
            if 0 <= n < coeff.shape[1]:
                d[j] += (-1) ** r * comb(4, r) / 6.0 * c64[:, n]
    c = (6.5 - np.arange(NB))[:, None] * d
    dt = d.astype(np.float32).T.reshape(G, P, NB).transpose(1, 0, 2).reshape(P, G * NB)
    ct = c.astype(np.float32).T.reshape(G, P, NB).transpose(1, 0, 2).reshape(P, G * NB)
    bt = np.broadcast_to(-np.arange(NB, dtype=np.float32), (P, NB))
    return np.ascontiguousarray(np.concatenate([dt, ct, bt], axis=1))


def kernel(x: np.ndarray, coeff: np.ndarray) -> np.ndarray:
    x = np.ascontiguousarray(x, dtype=np.float32)
    coeff = np.ascontiguousarray(coeff, dtype=np.float32)
    assert x.shape == (B_FULL, F) and coeff.shape == (F, 10)

    if "nc" not in _CACHE:
        _CACHE["nc"] = _build_nc()
    nc = _CACHE["nc"]

    tabs = _tables(coeff)

    in_maps = []
    for c in range(N_CORES):
        shard = np.ascontiguousarray(x[c * B_CORE : (c + 1) * B_CORE, :].T)
        in_maps.append({"xT": shard, "tabs": tabs})

    trace = os.environ.get("BSPLINE_TRACE", "0") == "1"
    res = run_bass_kernel_spmd(
        nc, in_maps, core_ids=list(range(N_CORES)), trace=trace
    )
    _CACHE["last_result"] = res

    y = np.empty((B_FULL, F), dtype=np.float32)
    for c in range(N_CORES):
        y[c * B_CORE : (c + 1) * B_CORE, :] = res.results[c]["yT"].T
    return y
